# revision 29
# baseline (speedup 1.0000x reference)
"""AdditiveAttention kernel for 8 TRN2 NeuronCores (data-parallel over batch).

reference:
  q_proj = query @ Wq_w.T + Wq_b          [B, S, D]
  k_proj = value @ Wk_w.T + Wk_b          [B, S, D]
  scores = tanh(q_proj + k_proj) @ V_w[0] + V_b[0]     [B, S]
  attn   = softmax(scores, axis=-1)
  ctx    = attn[..., None] * value
  returns (ctx, attn)

Device design (per core, 4 batches, everything in transposed [feature, token]
layout so every DMA descriptor is multi-KB contiguous):
  - combined[e, tok] accumulated in PSUM from 8 matmuls (4 k-tiles x {Wq,Wk})
    with pre-transposed bf16 activations (d on partitions).
  - tanh + per-partition bias fused on ScalarE -> bf16.
  - scores[1, tok] = V_w-weighted partition sum via PE matmul (M=1),
    deferred one chunk so the PE never waits on tanh.
  - softmax without max-subtraction (|scores| <= sum|V_w| + |V_b| ~ 23,
    exp is safe in f32; V_b cancels in softmax and is dropped). exp is
    fused into the PSUM->SBUF copy with accum_out partial sums.
  - softmax exp is broadcast to all partitions incrementally (GPSIMD
    partition_broadcast per chunk, hidden under the PE), normalized in
    place, and the context is one bf16 VectorE multiply per d-tile:
    ctxT[d, tok] = vT[d, tok] * attn[tok], written back transposed with
    16KB-contiguous descriptors. The host transposes/upcasts the output.
"""

import os
import sys
import types

sys.path.insert(0, "/opt/trn_rl_repo")

import numpy as np
import ml_dtypes

B, S, D = 32, 4096, 512
NCORES = 8
B_LOC = B // NCORES          # 4 batches per core
T = B_LOC * S                # 16384 tokens per core
P = 128
KD = D // P                  # 4 contraction tiles
ET = D // P                  # 4 output-feature tiles
HALF = 2048                  # activation load granularity (4KB descriptors)
CHUNK = 512                  # matmul moving free dim / PSUM bank
NCHUNK = S // CHUNK          # 8 scores chunks per batch
BF16 = ml_dtypes.bfloat16

LAST_EXEC_NS = None

_cache = {}


def _install_trace_shims():
    """Make trace=True work under axon in this container: the antenv here
    lacks axon_hooks, and upload_artifacts needs S3."""
    try:
        if "antenv.axon_hooks" not in sys.modules:
            from trn_agent_boot.trn_boot import _ntff_profile_via_ctypes

            hook = _ntff_profile_via_ctypes("/opt/axon/libaxon_pjrt.so")
            mod = types.ModuleType("antenv.axon_hooks")
            mod.get_axon_ntff_profile_hook = lambda: hook
            mod.set_axon_ntff_profile_hook = lambda h: None
            sys.modules["antenv.axon_hooks"] = mod
        import concourse.bass_utils as bu

        bu.upload_artifacts = lambda tmpdir: tmpdir
    except Exception:
        pass


def _build_nc():
    import concourse.tile as tile
    from concourse import bacc, mybir

    f32 = mybir.dt.float32
    bf16 = mybir.dt.bfloat16
    Act = mybir.ActivationFunctionType

    nc = bacc.Bacc(None, target_bir_lowering=False)

    qT = nc.declare_dram_parameter("qT", [D, T], bf16, isOutput=False)
    vT = nc.declare_dram_parameter("vT", [D, T], bf16, isOutput=False)
    wq = nc.declare_dram_parameter("wq", [D, D], bf16, isOutput=False)  # [d, e]
    wk = nc.declare_dram_parameter("wk", [D, D], bf16, isOutput=False)  # [d, e]
    bias = nc.declare_dram_parameter("bias", [P, ET], f32, isOutput=False)
    vw = nc.declare_dram_parameter("vw", [P, ET], bf16, isOutput=False)
    out_ctxT = nc.declare_dram_parameter("out_ctxT", [D, T], bf16, isOutput=True)
    out_attn = nc.declare_dram_parameter("out_attn", [B_LOC, S], f32, isOutput=True)

    with tile.TileContext(nc) as tc:
        with (
            tc.tile_pool(name="consts", bufs=1) as consts,
            tc.tile_pool(name="acts", bufs=2) as acts,
            tc.tile_pool(name="vtp", bufs=5) as vtp,
            tc.tile_pool(name="tanhp", bufs=3) as tanhp,
            tc.tile_pool(name="rows", bufs=2) as rows,
            tc.tile_pool(name="bcp", bufs=2) as bcp,
            tc.tile_pool(name="ctxp", bufs=2) as ctxp,
            tc.tile_pool(name="small", bufs=2) as small,
            tc.tile_pool(name="ps_qk", bufs=5, space="PSUM") as ps_qk,
            tc.tile_pool(name="ps_s", bufs=2, space="PSUM") as ps_s,
        ):
            wq_sb = consts.tile([P, KD, D], bf16)
            wk_sb = consts.tile([P, KD, D], bf16)
            wq_r = wq.rearrange("(kd p) e -> p kd e", p=P)
            wk_r = wk.rearrange("(kd p) e -> p kd e", p=P)

            qT_r = qT.rearrange("(kd p) t -> p kd t", p=P)
            vT_r = vT.rearrange("(kd p) t -> p kd t", p=P)
            ctxT_r = out_ctxT.rearrange("(kd p) t -> p kd t", p=P)

            # startup: load exactly what the first matmuls need, first
            nc.sync.dma_start(wq_sb[:, 0], wq_r[:, 0])
            nc.scalar.dma_start(wk_sb[:, 0], wk_r[:, 0])
            q0_sb = acts.tile([P, KD, HALF], bf16, tag="q")
            vt0_sb = vtp.tile([P, KD, HALF], bf16, tag="vt")
            nc.sync.dma_start(q0_sb[:, :, 0:CHUNK], qT_r[:, :, 0:CHUNK])
            nc.scalar.dma_start(vt0_sb[:, :, 0:CHUNK], vT_r[:, :, 0:CHUNK])
            for kd in range(1, KD):
                nc.sync.dma_start(wq_sb[:, kd], wq_r[:, kd])
                nc.scalar.dma_start(wk_sb[:, kd], wk_r[:, kd])
            bias_sb = consts.tile([P, ET], f32)
            nc.sync.dma_start(bias_sb[:], bias[:])
            vw_sb = consts.tile([P, ET], bf16)
            nc.scalar.dma_start(vw_sb[:], vw[:])
            nc.sync.dma_start(q0_sb[:, :, CHUNK:HALF], qT_r[:, :, CHUNK:HALF])
            nc.scalar.dma_start(vt0_sb[:, :, CHUNK:HALF], vT_r[:, :, CHUNK:HALF])

            for b in range(B_LOC):
                exp_row = rows.tile([1, S], bf16, tag="exp")
                sums_row = small.tile([1, NCHUNK], f32, tag="sums")
                # unnormalized exp, broadcast to all partitions incrementally
                # (per chunk, during the scores phase); normalized in place
                # once 1/sum is known, just before the context multiplies.
                exp_bc = bcp.tile([P, S], bf16, tag="exp_bc")

                def emit_scores(tanh_tile, g):
                    ps = ps_s.tile([1, CHUNK], f32, tag="s")
                    for e in range(ET):
                        nc.tensor.matmul(
                            ps[:],
                            lhsT=vw_sb[:, e : e + 1],
                            rhs=tanh_tile[:, e, :],
                            start=(e == 0),
                            stop=(e == ET - 1),
                        )
                    # exp fused into the PSUM->SBUF copy; partial sum via accum
                    nc.scalar.activation(
                        exp_row[:, g * CHUNK : (g + 1) * CHUNK],
                        ps[:],
                        Act.Exp,
                        accum_out=sums_row[:, g : g + 1],
                    )
                    # broadcast the unnormalized exp chunk while the PE works
                    nc.gpsimd.partition_broadcast(
                        exp_bc[:, g * CHUNK : (g + 1) * CHUNK],
                        exp_row[0:1, g * CHUNK : (g + 1) * CHUNK],
                    )


                pending = None
                vt_halves = []
                for h in range(S // HALF):
                    t0 = b * S + h * HALF
                    if b == 0 and h == 0:
                        q_sb, vt_sb = q0_sb, vt0_sb  # prefetched above
                    else:
                        q_sb = acts.tile([P, KD, HALF], bf16, tag="q")
                        vt_sb = vtp.tile([P, KD, HALF], bf16, tag="vt")
                        nc.sync.dma_start(q_sb[:], qT_r[:, :, t0 : t0 + HALF])
                        nc.sync.dma_start(vt_sb[:], vT_r[:, :, t0 : t0 + HALF])
                    vt_halves.append(vt_sb)

                    for j in range(HALF // CHUNK):
                        c0 = j * CHUNK
                        tanh_sb = tanhp.tile([P, ET, CHUNK], bf16, tag="tanh")
                        for e in range(ET):
                            pq = ps_qk.tile([P, CHUNK], f32, tag="qk")
                            for kd in range(KD):
                                nc.tensor.matmul(
                                    pq[:],
                                    lhsT=wq_sb[:, kd, e * P : (e + 1) * P],
                                    rhs=q_sb[:, kd, c0 : c0 + CHUNK],
                                    start=(kd == 0),
                                    stop=False,
                                )
                            for kd in range(KD):
                                nc.tensor.matmul(
                                    pq[:],
                                    lhsT=wk_sb[:, kd, e * P : (e + 1) * P],
                                    rhs=vt_sb[:, kd, c0 : c0 + CHUNK],
                                    start=False,
                                    stop=(kd == KD - 1),
                                )
                            nc.scalar.activation(
                                tanh_sb[:, e, :],
                                pq[:],
                                Act.Tanh,
                                bias=bias_sb[:, e : e + 1],
                            )
                        # scores matmuls deferred one chunk so the PE never
                        # waits on the tanh of the chunk it just produced
                        if pending is not None:
                            emit_scores(*pending)
                        pending = (tanh_sb, h * (HALF // CHUNK) + j)
                emit_scores(*pending)

                # softmax denominator for batch b; normalize the row in
                # place (bf16 tensor_scalar is fast even on one partition)
                total = small.tile([1, 1], f32, tag="total")
                nc.vector.reduce_sum(total[:], sums_row[:], axis=mybir.AxisListType.X)
                inv = small.tile([1, 1], f32, tag="inv")
                nc.vector.reciprocal(inv[:], total[:])
                inv128 = small.tile([P, 1], f32, tag="inv128")
                nc.gpsimd.partition_broadcast(inv128[:], inv[0:1, :])

                # context, transposed, bf16: normalize the broadcast tile in
                # place (fast bf16 tensor_scalar), then ctxT = vT * attn_bc
                for h in range(S // HALF):
                    nc.vector.tensor_scalar_mul(
                        exp_bc[:, h * HALF : (h + 1) * HALF],
                        exp_bc[:, h * HALF : (h + 1) * HALF],
                        inv128[:],
                    )
                    # 3D-AP multiplies (attn broadcast over kd via 0-stride
                    # AP), split in two so the stores start sooner
                    ctxT_h = ctxp.tile([P, KD, HALF], bf16, tag="ctxT")
                    for kd2 in range(0, KD, 2):
                        nc.vector.tensor_mul(
                            out=ctxT_h[:, kd2 : kd2 + 2, :],
                            in0=vt_halves[h][:, kd2 : kd2 + 2, :],
                            in1=exp_bc[
                                :, None, h * HALF : (h + 1) * HALF
                            ].to_broadcast((P, 2, HALF)),
                        )
                        for kd in (kd2, kd2 + 1):
                            nc.gpsimd.dma_start(
                                ctxT_r[
                                    :, kd, b * S + h * HALF : b * S + (h + 1) * HALF
                                ],
                                ctxT_h[:, kd, :],
                            )

                # partition 0 of the normalized broadcast tile IS the attn row
                nc.gpsimd.dma_start(out_attn[b : b + 1, :], exp_bc[0:1, :])

    nc.finalize()
    return nc


def _get_nc():
    if "nc" not in _cache:
        _cache["nc"] = _build_nc()
    return _cache["nc"]


def kernel(query, value, Wq_w, Wq_b, Wk_w, Wk_b, V_w, V_b):
    global LAST_EXEC_NS
    _install_trace_shims()
    from concourse.bass_utils import run_bass_kernel_spmd

    query = np.asarray(query, dtype=np.float32)
    value = np.asarray(value, dtype=np.float32)
    wq_t = np.ascontiguousarray(np.asarray(Wq_w, np.float32).T).astype(BF16)
    wk_t = np.ascontiguousarray(np.asarray(Wk_w, np.float32).T).astype(BF16)
    bias_sum = np.asarray(Wq_b, np.float32) + np.asarray(Wk_b, np.float32)
    bias_pack = np.ascontiguousarray(bias_sum.reshape(ET, P).T)  # [P, ET]
    vw_pack = np.ascontiguousarray(
        np.asarray(V_w, np.float32)[0].reshape(ET, P).T
    ).astype(BF16)  # [P, ET]

    in_maps = []
    for c in range(NCORES):
        qs = query[c * B_LOC : (c + 1) * B_LOC]  # [B_LOC, S, D]
        vs = value[c * B_LOC : (c + 1) * B_LOC]
        qT_h = np.ascontiguousarray(qs.transpose(2, 0, 1).reshape(D, T)).astype(BF16)
        vT_h = np.ascontiguousarray(vs.transpose(2, 0, 1).reshape(D, T)).astype(BF16)
        in_maps.append(
            {
                "qT": qT_h,
                "vT": vT_h,
                "wq": wq_t,
                "wk": wk_t,
                "bias": bias_pack,
                "vw": vw_pack,
            }
        )

    nc = _get_nc()
    trace = os.environ.get("KERNEL_TRACE") == "1"
    res = run_bass_kernel_spmd(nc, in_maps, core_ids=list(range(NCORES)), trace=trace)
    LAST_EXEC_NS = res.exec_time_ns

    # out_ctxT per core is [D, T]; transpose back on host
    ctx = np.concatenate(
        [
            res.results[c]["out_ctxT"]
            .astype(np.float32)
            .reshape(D, B_LOC, S)
            .transpose(1, 2, 0)
            for c in range(NCORES)
        ],
        axis=0,
    )
    attn = np.concatenate(
        [res.results[c]["out_attn"] for c in range(NCORES)], axis=0
    )
    return np.ascontiguousarray(ctx), attn


# revision 30
# speedup vs baseline: 1.1732x; 1.1732x over previous
"""AdditiveAttention kernel for 8 TRN2 NeuronCores (data-parallel over batch).

reference:
  q_proj = query @ Wq_w.T + Wq_b          [B, S, D]
  k_proj = value @ Wk_w.T + Wk_b          [B, S, D]
  scores = tanh(q_proj + k_proj) @ V_w[0] + V_b[0]     [B, S]
  attn   = softmax(scores, axis=-1)
  ctx    = attn[..., None] * value
  returns (ctx, attn)

Device design (per core, 4 batches, everything in transposed [feature, token]
layout so every DMA descriptor is multi-KB contiguous):
  - combined[e, tok] accumulated in PSUM from 8 matmuls (4 k-tiles x {Wq,Wk})
    with pre-transposed bf16 activations (d on partitions).
  - tanh + per-partition bias fused on ScalarE -> bf16.
  - scores[1, tok] = V_w-weighted partition sum via PE matmul (M=1),
    deferred one chunk so the PE never waits on tanh.
  - softmax without max-subtraction (|scores| <= sum|V_w| + |V_b| ~ 23,
    exp is safe in f32; V_b cancels in softmax and is dropped). exp is
    fused into the PSUM->SBUF copy with accum_out partial sums.
  - softmax exp is broadcast to all partitions incrementally (GPSIMD
    partition_broadcast per chunk, hidden under the PE), normalized in
    place, and the context is one bf16 VectorE multiply per d-tile:
    ctxT[d, tok] = vT[d, tok] * attn[tok], written back transposed with
    16KB-contiguous descriptors. The host transposes/upcasts the output.
"""

import os
import sys
import types

sys.path.insert(0, "/opt/trn_rl_repo")

import numpy as np
import ml_dtypes

B, S, D = 32, 4096, 512
NCORES = 8
B_LOC = B // NCORES          # 4 batches per core
T = B_LOC * S                # 16384 tokens per core
P = 128
KD = D // P                  # 4 contraction tiles
ET = D // P                  # 4 output-feature tiles
HALF = 2048                  # activation load granularity (4KB descriptors)
CHUNK = 512                  # matmul moving free dim / PSUM bank
NCHUNK = S // CHUNK          # 8 scores chunks per batch
BF16 = ml_dtypes.bfloat16

LAST_EXEC_NS = None

_cache = {}


def _install_trace_shims():
    """Make trace=True work under axon in this container: the antenv here
    lacks axon_hooks, and upload_artifacts needs S3."""
    try:
        if "antenv.axon_hooks" not in sys.modules:
            from trn_agent_boot.trn_boot import _ntff_profile_via_ctypes

            hook = _ntff_profile_via_ctypes("/opt/axon/libaxon_pjrt.so")
            mod = types.ModuleType("antenv.axon_hooks")
            mod.get_axon_ntff_profile_hook = lambda: hook
            mod.set_axon_ntff_profile_hook = lambda h: None
            sys.modules["antenv.axon_hooks"] = mod
        import concourse.bass_utils as bu

        bu.upload_artifacts = lambda tmpdir: tmpdir
    except Exception:
        pass


def _build_nc():
    import concourse.tile as tile
    from concourse import bacc, mybir

    f32 = mybir.dt.float32
    bf16 = mybir.dt.bfloat16
    Act = mybir.ActivationFunctionType

    nc = bacc.Bacc(None, target_bir_lowering=False)

    qT = nc.declare_dram_parameter("qT", [D, T], bf16, isOutput=False)
    vT = nc.declare_dram_parameter("vT", [D, T], bf16, isOutput=False)
    wq = nc.declare_dram_parameter("wq", [D, D], bf16, isOutput=False)  # [d, e]
    wk = nc.declare_dram_parameter("wk", [D, D], bf16, isOutput=False)  # [d, e]
    bias = nc.declare_dram_parameter("bias", [P, ET], f32, isOutput=False)
    vw = nc.declare_dram_parameter("vw", [P, ET], bf16, isOutput=False)
    out_ctxT = nc.declare_dram_parameter("out_ctxT", [D, T], bf16, isOutput=True)
    out_attn = nc.declare_dram_parameter("out_attn", [B_LOC, S], f32, isOutput=True)

    with tile.TileContext(nc) as tc:
        with (
            tc.tile_pool(name="consts", bufs=1) as consts,
            tc.tile_pool(name="acts", bufs=2) as acts,
            tc.tile_pool(name="vtp", bufs=5) as vtp,
            tc.tile_pool(name="tanhp", bufs=3) as tanhp,
            tc.tile_pool(name="rows", bufs=2) as rows,
            tc.tile_pool(name="bcp", bufs=2) as bcp,
            tc.tile_pool(name="ctxp", bufs=2) as ctxp,
            tc.tile_pool(name="small", bufs=2) as small,
            tc.tile_pool(name="ps_qk", bufs=5, space="PSUM") as ps_qk,
            tc.tile_pool(name="ps_s", bufs=2, space="PSUM") as ps_s,
        ):
            wq_sb = consts.tile([P, KD, D], bf16)
            wk_sb = consts.tile([P, KD, D], bf16)
            wq_r = wq.rearrange("(kd p) e -> p kd e", p=P)
            wk_r = wk.rearrange("(kd p) e -> p kd e", p=P)

            qT_r = qT.rearrange("(kd p) t -> p kd t", p=P)
            vT_r = vT.rearrange("(kd p) t -> p kd t", p=P)
            ctxT_r = out_ctxT.rearrange("(kd p) t -> p kd t", p=P)

            # startup: load exactly what the first matmuls need, first
            nc.sync.dma_start(wq_sb[:, 0], wq_r[:, 0])
            nc.scalar.dma_start(wk_sb[:, 0], wk_r[:, 0])
            q0_sb = acts.tile([P, KD, HALF], bf16, tag="q")
            vt0_sb = vtp.tile([P, KD, HALF], bf16, tag="vt")
            nc.sync.dma_start(q0_sb[:, :, 0:CHUNK], qT_r[:, :, 0:CHUNK])
            nc.scalar.dma_start(vt0_sb[:, :, 0:CHUNK], vT_r[:, :, 0:CHUNK])
            for kd in range(1, KD):
                nc.sync.dma_start(wq_sb[:, kd], wq_r[:, kd])
                nc.scalar.dma_start(wk_sb[:, kd], wk_r[:, kd])
            bias_sb = consts.tile([P, ET], f32)
            nc.sync.dma_start(bias_sb[:], bias[:])
            vw_sb = consts.tile([P, ET], bf16)
            nc.scalar.dma_start(vw_sb[:], vw[:])
            nc.sync.dma_start(q0_sb[:, :, CHUNK:HALF], qT_r[:, :, CHUNK:HALF])
            nc.scalar.dma_start(vt0_sb[:, :, CHUNK:HALF], vT_r[:, :, CHUNK:HALF])

            for b in range(B_LOC):
                exp_row = rows.tile([1, S], bf16, tag="exp")
                sums_row = small.tile([1, NCHUNK], f32, tag="sums")
                # unnormalized exp, broadcast to all partitions incrementally
                # (per chunk, during the scores phase); normalized in place
                # once 1/sum is known, just before the context multiplies.
                exp_bc = bcp.tile([P, S], bf16, tag="exp_bc")

                def emit_scores(tanh_tile, g):
                    ps = ps_s.tile([1, CHUNK], f32, tag="s")
                    for e in range(ET):
                        nc.tensor.matmul(
                            ps[:],
                            lhsT=vw_sb[:, e : e + 1],
                            rhs=tanh_tile[:, e, :],
                            start=(e == 0),
                            stop=(e == ET - 1),
                        )
                    # exp fused into the PSUM->SBUF copy; partial sum via accum
                    nc.scalar.activation(
                        exp_row[:, g * CHUNK : (g + 1) * CHUNK],
                        ps[:],
                        Act.Exp,
                        accum_out=sums_row[:, g : g + 1],
                    )
                    # broadcast the unnormalized exp chunk while the PE works
                    nc.gpsimd.partition_broadcast(
                        exp_bc[:, g * CHUNK : (g + 1) * CHUNK],
                        exp_row[0:1, g * CHUNK : (g + 1) * CHUNK],
                    )


                pending = None
                vt_halves = []
                for h in range(S // HALF):
                    t0 = b * S + h * HALF
                    if b == 0 and h == 0:
                        q_sb, vt_sb = q0_sb, vt0_sb  # prefetched above
                    else:
                        q_sb = acts.tile([P, KD, HALF], bf16, tag="q")
                        vt_sb = vtp.tile([P, KD, HALF], bf16, tag="vt")
                        nc.sync.dma_start(q_sb[:], qT_r[:, :, t0 : t0 + HALF])
                        nc.sync.dma_start(vt_sb[:], vT_r[:, :, t0 : t0 + HALF])
                    vt_halves.append(vt_sb)

                    for j in range(HALF // CHUNK):
                        c0 = j * CHUNK
                        tanh_sb = tanhp.tile([P, ET, CHUNK], bf16, tag="tanh")
                        for e in range(ET):
                            pq = ps_qk.tile([P, CHUNK], f32, tag="qk")
                            for kd in range(KD):
                                nc.tensor.matmul(
                                    pq[:],
                                    lhsT=wq_sb[:, kd, e * P : (e + 1) * P],
                                    rhs=q_sb[:, kd, c0 : c0 + CHUNK],
                                    start=(kd == 0),
                                    stop=False,
                                )
                            for kd in range(KD):
                                nc.tensor.matmul(
                                    pq[:],
                                    lhsT=wk_sb[:, kd, e * P : (e + 1) * P],
                                    rhs=vt_sb[:, kd, c0 : c0 + CHUNK],
                                    start=False,
                                    stop=(kd == KD - 1),
                                )
                            nc.scalar.activation(
                                tanh_sb[:, e, :],
                                pq[:],
                                Act.Tanh,
                                bias=bias_sb[:, e : e + 1],
                            )
                        # scores matmuls deferred one chunk so the PE never
                        # waits on the tanh of the chunk it just produced
                        if pending is not None:
                            emit_scores(*pending)
                        pending = (tanh_sb, h * (HALF // CHUNK) + j)
                emit_scores(*pending)

                # softmax denominator for batch b; normalize the row in
                # place (bf16 tensor_scalar is fast even on one partition)
                total = small.tile([1, 1], f32, tag="total")
                nc.vector.reduce_sum(total[:], sums_row[:], axis=mybir.AxisListType.X)
                inv = small.tile([1, 1], f32, tag="inv")
                nc.vector.reciprocal(inv[:], total[:])
                inv128 = small.tile([P, 1], f32, tag="inv128")
                nc.gpsimd.partition_broadcast(inv128[:], inv[0:1, :])

                # context, transposed, bf16: normalize the broadcast tile in
                # place (fast bf16 tensor_scalar), then ctxT = vT * attn_bc
                for h in range(S // HALF):
                    nc.vector.tensor_scalar_mul(
                        exp_bc[:, h * HALF : (h + 1) * HALF],
                        exp_bc[:, h * HALF : (h + 1) * HALF],
                        inv128[:],
                    )
                    # 3D-AP multiplies (attn broadcast over kd via 0-stride
                    # AP), split in two so the stores start sooner
                    ctxT_h = ctxp.tile([P, KD, HALF], bf16, tag="ctxT")
                    for kd2 in range(0, KD, 2):
                        nc.vector.tensor_mul(
                            out=ctxT_h[:, kd2 : kd2 + 2, :],
                            in0=vt_halves[h][:, kd2 : kd2 + 2, :],
                            in1=exp_bc[
                                :, None, h * HALF : (h + 1) * HALF
                            ].to_broadcast((P, 2, HALF)),
                        )
                        for kd in (kd2, kd2 + 1):
                            # in the last batch the load rings are idle: split
                            # the tail stores across two issue engines
                            eng = (
                                nc.sync if (b == B_LOC - 1 and kd % 2 == 0)
                                else nc.gpsimd
                            )
                            eng.dma_start(
                                ctxT_r[
                                    :, kd, b * S + h * HALF : b * S + (h + 1) * HALF
                                ],
                                ctxT_h[:, kd, :],
                            )

                # partition 0 of the normalized broadcast tile IS the attn row
                nc.gpsimd.dma_start(out_attn[b : b + 1, :], exp_bc[0:1, :])

    nc.finalize()
    return nc


def _get_nc():
    if "nc" not in _cache:
        _cache["nc"] = _build_nc()
    return _cache["nc"]


def kernel(query, value, Wq_w, Wq_b, Wk_w, Wk_b, V_w, V_b):
    global LAST_EXEC_NS
    _install_trace_shims()
    from concourse.bass_utils import run_bass_kernel_spmd

    query = np.asarray(query, dtype=np.float32)
    value = np.asarray(value, dtype=np.float32)
    wq_t = np.ascontiguousarray(np.asarray(Wq_w, np.float32).T).astype(BF16)
    wk_t = np.ascontiguousarray(np.asarray(Wk_w, np.float32).T).astype(BF16)
    bias_sum = np.asarray(Wq_b, np.float32) + np.asarray(Wk_b, np.float32)
    bias_pack = np.ascontiguousarray(bias_sum.reshape(ET, P).T)  # [P, ET]
    vw_pack = np.ascontiguousarray(
        np.asarray(V_w, np.float32)[0].reshape(ET, P).T
    ).astype(BF16)  # [P, ET]

    in_maps = []
    for c in range(NCORES):
        qs = query[c * B_LOC : (c + 1) * B_LOC]  # [B_LOC, S, D]
        vs = value[c * B_LOC : (c + 1) * B_LOC]
        qT_h = np.ascontiguousarray(qs.transpose(2, 0, 1).reshape(D, T)).astype(BF16)
        vT_h = np.ascontiguousarray(vs.transpose(2, 0, 1).reshape(D, T)).astype(BF16)
        in_maps.append(
            {
                "qT": qT_h,
                "vT": vT_h,
                "wq": wq_t,
                "wk": wk_t,
                "bias": bias_pack,
                "vw": vw_pack,
            }
        )

    nc = _get_nc()
    trace = os.environ.get("KERNEL_TRACE") == "1"
    res = run_bass_kernel_spmd(nc, in_maps, core_ids=list(range(NCORES)), trace=trace)
    LAST_EXEC_NS = res.exec_time_ns

    # out_ctxT per core is [D, T]; transpose back on host
    ctx = np.concatenate(
        [
            res.results[c]["out_ctxT"]
            .astype(np.float32)
            .reshape(D, B_LOC, S)
            .transpose(1, 2, 0)
            for c in range(NCORES)
        ],
        axis=0,
    )
    attn = np.concatenate(
        [res.results[c]["out_attn"] for c in range(NCORES)], axis=0
    )
    return np.ascontiguousarray(ctx), attn
